# revision 32
# baseline (speedup 1.0000x reference)
"""Tensor-parallel Llama attention for 8 TRN2 NeuronCores.

Sharding: core d handles batch d//4 and q-head group g = d%4 (q heads
4g..4g+3, kv head g — GQA group-aligned so each core needs exactly one
kv head).  Wq/Wk/Wv are row-sharded, Wo column-sharded; the per-batch
partial o_proj outputs of 4 cores are summed on the host.

Device layouts (prepared host-side, bf16):
  hsT  [16,128,S]   hidden_states[b].T, HID on partitions in 16 chunks
  wqT  [16,128,512] Wq_shard.T          wkT/wvT [16,128,128]
  woT  [4,128,2048] Wo_shard.T (4 contraction chunks of the 512 local dims)
  cosT/sinT [128,S] RoPE tables in [head_dim, seq] layout
  mask [4,128,512]  0/1 causal masks for the 4 diagonal-block phases

Schedule (v4): half-chunks of hs0/wk/wv DMA first so the k-projection
starts as early as possible; K+V projections run contraction-chunk-
outer with 8 PSUM accumulators streaming right behind the hsT DMA.
Attention probs are query-aligned and tree-accumulated 4:1 on the DVE
in bf16, so the softmax-denominator ones-matmuls only stream every
4th tile through the PE (quarter-rate f32 sums — this cut PE busy by
~25us vs per-tile sum matmuls).  qproj(2..3) chains and all oproj row
tiles are interleaved INTO the attention tile loops as per-head PE
filler (fillers[h], emitted after each head's first two scores), and
each group carries a post-filler (key 4) after its last head so the
next group's fin broadcast-matmuls — which wait on the ln/exp
reciprocal chain — always have PE cover.  Rope rotate chains slot
into per-head DVE gaps (dve_extra) so masks/tree-adds aren't queued
behind them.  Diagonal blocks are width-trimmed to the causal
triangle with a 2-tile score lookahead.  Denominators are exp(-ln(s))
on ACT; pv*(1/sum) is fused into one scalar_tensor_tensor.  o_proj
stages whole 128x2048 bf16 rows, one DMA per row tile; partial
outputs are summed on the host in f32.  (fp8/DoubleRow was evaluated
and rejected: e4m3's ~3% GEMM error exceeds the 2e-2 gate on every
path except qproj, and even there leaves no margin.)
"""

import sys

sys.path.insert(0, "/opt/trn_rl_repo")

import numpy as np
import ml_dtypes

B, S, HID = 2, 2048, 2048
NH, NKV, HD = 16, 4, 128
THETA = 10000.0
NCORES = 8
HPC = 4            # q heads per core
QDIM = HPC * HD    # 512 local q dims
KT = HID // 128    # 16 contraction chunks
SB = S // 512      # 4 column groups of 512
ST = S // 128      # 16 row tiles of 128

_CACHE = {}


def _patch_tile_drain():
    """This walrus build caps sync waits per CTRL instruction below what the
    stock Tile kernel-tail drain carries; split them into single-wait NOPs."""
    import bass_rust
    import concourse.tile as tile
    from concourse.tile import ScopedClock

    if getattr(tile.TileContext, "_drain_split_patched", False):
        return

    def _split_drain_and_barrier(self, tick_clock, wait_clock):
        ticks = list(tick_clock.global_clock)
        for i, v in enumerate(ticks):
            if v > 0:
                single = [0] * len(ticks)
                single[i] = v
                nop = self.nc.sync.nop(nofuse=True, hint=f"drain_wait_{i}")
                wait_clock.add_sem_waits(
                    nop.ins, ScopedClock({None: bass_rust.VectorClock(single)})
                )
        self.nc.sync.drain()
        self.nc.all_engine_barrier()
        assert self.sems is not None
        popped = self.nc._tile_sem_poison_stack.pop()
        assert popped is self._sem_poison
        self.nc.clear_and_free_semaphores(list(self.sems.allocated().values()))
        self.nc.all_engine_barrier()

    tile.TileContext._drain_and_barrier = _split_drain_and_barrier
    tile.TileContext._drain_split_patched = True


def _legalize_waits(nc, max_waits=1):
    """This walrus build rejects instructions carrying more than ~2 sync
    waits.  Hoist the excess onto single-wait NOPs inserted just before the
    instruction in its block (same engine => same instruction stream, so
    the waits still complete before the op issues)."""
    import concourse.mybir as mybir

    n_split = 0
    for block in nc.m.functions[0].blocks:
        insts = list(block.instructions)
        out = []
        for inst in insts:
            si = getattr(inst, "sync_info", None)
            if si is not None and si.on_wait and len(si.on_wait) > max_waits:
                waits = list(si.on_wait)
                keep = waits[:max_waits]
                for j, w in enumerate(waits[max_waits:]):
                    out.append(
                        mybir.InstNoOp(
                            name=f"{inst.name}_hw{j}",
                            engine=inst.engine,
                            bass_nofuse=True,
                            sync_info=mybir.SyncInfo(on_wait=[w], on_update=[]),
                        )
                    )
                si.on_wait = keep
                n_split += 1
            out.append(inst)
        block.instructions = out
    return n_split


def _build_nc():
    import concourse.bass as bass
    import concourse.mybir as mybir
    import concourse.tile as tile
    from concourse.masks import make_identity

    _patch_tile_drain()

    bf = mybir.dt.bfloat16
    f32 = mybir.dt.float32
    Exp = mybir.ActivationFunctionType.Exp
    Ln = mybir.ActivationFunctionType.Ln
    Mult = mybir.AluOpType.mult

    nc = bass.Bass()
    hsT = nc.declare_dram_parameter("hsT", [KT, 128, S], bf, isOutput=False)
    wqT = nc.declare_dram_parameter("wqT", [128, KT * QDIM], bf, isOutput=False)
    wkT = nc.declare_dram_parameter("wkT", [128, KT * HD], bf, isOutput=False)
    wvT = nc.declare_dram_parameter("wvT", [128, KT * HD], bf, isOutput=False)
    woT = nc.declare_dram_parameter("woT", [128, 4 * HID], bf, isOutput=False)
    cosT = nc.declare_dram_parameter("cosT", [128, S], bf, isOutput=False)
    sinT = nc.declare_dram_parameter("sinT", [128, S], bf, isOutput=False)
    mask = nc.declare_dram_parameter("mask", [128, 4 * 512], bf, isOutput=False)
    out = nc.declare_dram_parameter("out", [S, HID], bf, isOutput=True)

    inv_sqrt_d = 1.0 / float(np.sqrt(HD))

    with tile.TileContext(nc) as tc:
        with (
            tc.tile_pool(name="resid", bufs=1) as resid,
            tc.tile_pool(name="probs", bufs=10) as probs_pool,
            tc.tile_pool(name="rot", bufs=2) as rot_pool,
            tc.tile_pool(name="rcp", bufs=4) as rcp_pool,
            tc.tile_pool(name="ostage", bufs=3) as ostage_pool,
        ):
            hs_sb = resid.tile([128, KT * S], bf)
            wq_sb = resid.tile([128, KT * QDIM], bf)
            wk_sb = resid.tile([128, KT * HD], bf)
            wv_sb = resid.tile([128, KT * HD], bf)
            wo_sb = resid.tile([128, 4 * HID], bf)
            cos_sb = resid.tile([128, S], bf)
            sin_sb = resid.tile([128, S], bf)
            mask_sb = resid.tile([128, 4 * 512], bf)
            ones_sb = resid.tile([128, 1], bf)
            ones128 = resid.tile([128, 128], bf)
            ident = resid.tile([128, 128], bf)
            qT_sb = resid.tile([128, HPC * S], bf)
            kT_sb = resid.tile([128, S], bf)
            vT_sb = resid.tile([128, S], bf)
            vn_sb = resid.tile([128, S], bf)
            at_sb = resid.tile([128, HPC * S], bf)

            # ---- loads: hs chunk 0 goes FIRST so the k-projection's first
            # matmul can issue at ~2.5us (wk arrives right behind it); the
            # wq/cos/sin/mask transfers are slotted into the hs stream at
            # the point their consumers need them ----
            QC = 4 * QDIM
            # half-chunks of hs0/wk/wv go first so the k-projection's
            # opening matmuls start as soon as ~512KB has landed; the
            # wq/cos/sin/mask transfers are slotted into the hs stream at
            # the point their consumers need them
            nc.sync.dma_start(hs_sb[:, 0:1024], hsT[0][:, 0:1024])
            nc.sync.dma_start(wk_sb[:, 0:1024], wkT[:, 0:1024])
            nc.sync.dma_start(wv_sb[:, 0:1024], wvT[:, 0:1024])
            nc.sync.dma_start(hs_sb[:, 1024:S], hsT[0][:, 1024:S])
            nc.sync.dma_start(wk_sb[:, 1024:2048], wkT[:, 1024:2048])
            nc.sync.dma_start(wv_sb[:, 1024:2048], wvT[:, 1024:2048])
            for kk in range(1, 8):
                nc.sync.dma_start(hs_sb[:, kk * S:(kk + 1) * S], hsT[kk])
            nc.sync.dma_start(wq_sb[:, 0:QC], wqT[:, 0:QC])
            for kk in range(8, 12):
                nc.sync.dma_start(hs_sb[:, kk * S:(kk + 1) * S], hsT[kk])
            nc.sync.dma_start(cos_sb[:], cosT[:])
            nc.sync.dma_start(sin_sb[:], sinT[:])
            for kk in range(12, KT):
                nc.sync.dma_start(hs_sb[:, kk * S:(kk + 1) * S], hsT[kk])
            nc.sync.dma_start(mask_sb[:], mask[:])
            for c in range(1, 4):
                nc.sync.dma_start(wq_sb[:, c * QC:(c + 1) * QC], wqT[:, c * QC:(c + 1) * QC])
            nc.sync.dma_start(wo_sb[:], woT[:])
            nc.gpsimd.memset(ones_sb[:], 1.0)
            nc.gpsimd.memset(ones128[:], 1.0)
            make_identity(nc, ident[:])

            def rope_rotate(dst, sg):
                # in-place rope of dst (sbuf bf16 [128,512]); sg picks the
                # cos/sin column group.
                cs = cos_sb[:, sg * 512:(sg + 1) * 512]
                sn = sin_sb[:, sg * 512:(sg + 1) * 512]
                rot = rot_pool.tile([128, 512], bf, name="rot", tag="rot")
                nc.vector.tensor_scalar_mul(rot[0:64, :], dst[64:128, :], -1.0)
                nc.vector.tensor_copy(rot[64:128, :], dst[0:64, :])
                nc.vector.tensor_mul(dst, dst, cs)
                nc.vector.tensor_mul(rot[:], rot[:], sn)
                nc.vector.tensor_add(dst, dst, rot[:])


            # ---- k/v projections, contraction-chunk outer ----
            with tc.tile_pool(name="ldps", bufs=8, space="PSUM") as ldps:
                ldk = [ldps.tile([128, 512], f32, name=f"ldk{sg}", tag="ld")
                       for sg in range(SB)]
                ldv = [ldps.tile([128, 512], f32, name=f"ldv{sg}", tag="ld")
                       for sg in range(SB)]
                for kk in range(KT):
                    for sg in range(SB):
                        nc.tensor.matmul(
                            ldk[sg][:],
                            wk_sb[:, kk * HD: kk * HD + 128],
                            hs_sb[:, kk * S + sg * 512: kk * S + sg * 512 + 512],
                            start=(kk == 0), stop=(kk == KT - 1),
                        )
                        nc.tensor.matmul(
                            ldv[sg][:],
                            wv_sb[:, kk * HD: kk * HD + 128],
                            hs_sb[:, kk * S + sg * 512: kk * S + sg * 512 + 512],
                            start=(kk == 0), stop=(kk == KT - 1),
                        )
                # v copies on ACT first so the PE transposes can start while
                # the DVE is still busy with rope(k).
                for sg in range(SB):
                    nc.scalar.copy(vT_sb[:, sg * 512:(sg + 1) * 512], ldv[sg][:])
                for sg in range(SB):
                    nc.vector.tensor_copy(kT_sb[:, sg * 512:(sg + 1) * 512], ldk[sg][:])
                # attn(0) only touches j-tiles 0-3 (= sg0): rotate that one
                # now, defer sg1-3 until after qproj(0)'s rotates
                rope_rotate(kT_sb[:, 0:512], 0)

            with (
                tc.tile_pool(name="tr_ps", bufs=3, space="PSUM") as tr_ps,
                tc.tile_pool(name="pv_ps", bufs=4, space="PSUM") as pv_ps,
                tc.tile_pool(name="sm_ps", bufs=1, space="PSUM") as sm_ps,
            ):
                def vtrans(tj0, tj1):
                    # ---- v back to natural [s, d] layout via PE transpose ----
                    for tj in range(tj0, tj1):
                        tp = tr_ps.tile([128, 128], bf, name="tp", tag="tr")
                        nc.tensor.transpose(tp[:], vT_sb[:, tj * 128:(tj + 1) * 128], ident[:])
                        nc.vector.tensor_copy(vn_sb[:, tj * 128:(tj + 1) * 128], tp[:])

                def qsl(gi, h):
                    return qT_sb[:, h * S + gi * 512: h * S + gi * 512 + 512]

                def qproj_head(gi, h):
                    # one 16-matmul accumulation chain; psum copy on ACT so
                    # the DVE stays free for rope/mask/accumulate work
                    ps = tr_ps.tile([128, 512], f32, name="qps", tag="tr")
                    for kk in range(KT):
                        nc.tensor.matmul(
                            ps[:],
                            wq_sb[:, kk * QDIM + h * 128: kk * QDIM + (h + 1) * 128],
                            hs_sb[:, kk * S + gi * 512: kk * S + gi * 512 + 512],
                            start=(kk == 0), stop=(kk == KT - 1),
                        )
                    sl = qsl(gi, h)
                    nc.scalar.copy(sl, ps[:])
                    return sl

                def attn(gi, pre=None, fillers=None, dve_extra=None,
                         fillq=None, stride=4):
                    fillq = fillq if fillq is not None else []
                    # Per-head tiles, query-aligned: tile t's valid columns
                    # are [ioff:512] (ioff=128p on the 4 diagonal tiles, 0
                    # elsewhere).  Probs tree-accumulate 4:1 on the DVE in
                    # bf16 and only the group-of-4 roots stream through the
                    # ones-matmul into smp (quarter-rate f32 sums).
                    # pre() is emitted after head 0's first two scores;
                    # fillers[h] (PE chains) after head h's first two
                    # scores; dve_extra[h] (DVE work) after head h's tiles.
                    tiles = []
                    for tj in range(4 * gi):
                        tiles.append((tj, 0, None))
                    for p in range(4):
                        tiles.append((4 * gi + p, 128 * p, p))
                    ntile = len(tiles)
                    ngrp = gi + 1

                    smp = sm_ps.tile([128, 512], f32, name="smp", tag="sm")
                    pvs = []

                    def emit_head(h):
                        qh = qsl(gi, h)
                        pv = pv_ps.tile([128, 512], f32, name="pv", tag="pv")
                        pvs.append(pv)
                        # software pipeline with one-tile lookahead: emit
                        # score(t+1) before pv(t) so exp overlaps PE.
                        pbs = [None] * ntile
                        # sum matmuls are deferred ~3 tiles past their
                        # group's last DVE tree-add so the PE never waits
                        # on the add chain
                        pend_sums = []

                        def flush_sums(now_t):
                            while pend_sums and (
                                now_t is None or now_t >= pend_sums[0][1] + 6
                            ):
                                g, t0 = pend_sums.pop(0)
                                nc.tensor.matmul(
                                    smp[32 * h:32 * h + 1, 0:512],
                                    ones_sb[:], pbs[t0][:, 0:512],
                                    start=(g == 0), stop=(g == ngrp - 1),
                                    skip_group_check=True,
                                    tile_position=(0, 32 * h),
                                )

                        def emit_score(t):
                            tj, ioff, p = tiles[t]
                            sc = tr_ps.tile([128, 512], f32, name="sc", tag="tr")
                            nc.tensor.matmul(
                                sc[:, ioff:512],
                                kT_sb[:, tj * 128:(tj + 1) * 128],
                                qh[:, ioff:512],
                                start=True, stop=True,
                            )
                            pb = probs_pool.tile([128, 512], bf, name="pb")
                            nc.scalar.activation(
                                pb[:, ioff:512], sc[:, ioff:512], Exp,
                                scale=inv_sqrt_d,
                            )
                            if p is not None:
                                # causal trim on the (idle) gpsimd: in the
                                # query-aligned slice the condition is just
                                # col >= partition, and the gpsimd queue is
                                # empty so this never waits behind the DVE's
                                # tree-adds/ropes in the exp->pv chain
                                nc.gpsimd.affine_select(
                                    out=pb[:, ioff:512], in_=pb[:, ioff:512],
                                    compare_op=mybir.AluOpType.is_ge,
                                    fill=0.0, base=0,
                                    pattern=[[1, 512 - ioff]],
                                    channel_multiplier=-1,
                                )
                            pbs[t] = pb

                        def emit_sumpv(t):
                            tj, ioff, p = tiles[t]
                            nc.tensor.matmul(
                                pv[:, ioff:512],
                                vn_sb[:, tj * 128:(tj + 1) * 128],
                                pbs[t][:, ioff:512],
                                start=(t == 0), stop=(t == ntile - 1),
                                skip_group_check=True,
                            )
                            g, r = divmod(t, 4)
                            t0 = 4 * g
                            if r == 1:
                                io = tiles[t][1]
                                nc.vector.tensor_add(
                                    pbs[t0][:, io:512], pbs[t0][:, io:512],
                                    pbs[t][:, io:512],
                                )
                            elif r == 3:
                                io = tiles[t][1]
                                nc.vector.tensor_add(
                                    pbs[t0 + 2][:, io:512],
                                    pbs[t0 + 2][:, io:512], pbs[t][:, io:512],
                                )
                                io2 = tiles[t0 + 2][1]
                                nc.vector.tensor_add(
                                    pbs[t0][:, io2:512], pbs[t0][:, io2:512],
                                    pbs[t0 + 2][:, io2:512],
                                )
                                pend_sums.append((g, t0))

                        emit_score(0)
                        if ntile > 1:
                            emit_score(1)
                        if h == 0 and pre is not None:
                            pre()
                        if fillers is not None:
                            for fl in fillers.get(h, []):
                                fl()
                        for t in range(ntile):
                            if t + 2 < ntile:
                                emit_score(t + 2)
                            emit_sumpv(t)
                            flush_sums(t)
                            # fine-grained PE filler inside the tile loop:
                            # the ACT exp runs ~130ns/tile slower than the
                            # PE's score+pv work, and the 3-slot score psum
                            # ring caps its lookahead, so without these the
                            # PE stalls on pb mid-head.
                            if fillq and t >= 1 and t % stride == stride - 1:
                                fillq.pop(0)()
                        flush_sums(None)
                        if dve_extra is not None:
                            for fn in dve_extra.get(h, []):
                                fn()

                    for h in range(HPC):
                        emit_head(h)
                    # drain any filler units the tile loops didn't consume
                    while fillq:
                        fillq.pop(0)()
                    # batched 1/sums for all 4 heads (rows 32h of smp) as
                    # exp(-ln(s)) on ACT: ln/exp/copy share one act table, and
                    # this is ~2.6us cheaper than the DVE reciprocal.  The
                    # per-head broadcast+normalize follows in fin(gi), which
                    # the caller schedules behind PE filler work.
                    lns = rcp_pool.tile([128, 512], f32, name="lns", tag="rcp")
                    nc.scalar.activation(lns[:], smp[:], Ln)
                    rcpb = rcp_pool.tile([128, 512], bf, name="rcpb", tag="rcpb")
                    nc.scalar.activation(rcpb[:], lns[:], Exp, scale=-1.0)
                    # post-filler slot (key 4): PE work emitted after the
                    # last head so this group's own fin broadcast matmuls
                    # (which wait on the ln/exp chain) have PE cover.
                    if fillers is not None:
                        for fl in fillers.get(4, []):
                            fl()
                    # normalize in-group: by now the reciprocals are ready
                    # (post-filler covered the ACT latency), the tr_ps ring
                    # has free slots, and finishing here releases the pv
                    # banks before the next group allocates them.
                    fin(gi, pvs, rcpb)

                def fin(gi, pvs, rcpb):
                    for h in range(HPC):
                        bc = tr_ps.tile([128, 512], f32, name="bc", tag="tr")
                        nc.tensor.matmul(
                            bc[:], ones128[32 * h:32 * h + 1, :],
                            rcpb[32 * h:32 * h + 1, :],
                            start=True, stop=True,
                            tile_position=(32 * h, 0),
                        )
                        bcs = rcp_pool.tile([128, 512], bf, name="bcs", tag="bcs")
                        nc.scalar.copy(bcs[:], bc[:])
                        a_sl = at_sb[:, h * S + gi * 512: h * S + gi * 512 + 512]
                        nc.vector.scalar_tensor_tensor(
                            a_sl, pvs[h][:], 1.0, bcs[:], Mult, Mult
                        )

                def oproj_eg(st, eg, cell):
                    # one eg-chain of an output row tile (4 matmuls + copy);
                    # the ostage tile is allocated on eg 0 and the row DMA
                    # issued on eg 3, so these units can be sprinkled into
                    # the attention tile loops as fine-grained PE filler
                    if eg == 0:
                        cell[0] = ostage_pool.tile([128, HID], bf, name="ostage")
                    ostage = cell[0]
                    last = st == 15
                    ps = tr_ps.tile([128, 512], f32, name="ops", tag="tr")
                    for h in range(HPC):
                        nc.tensor.matmul(
                            ps[:],
                            at_sb[:, h * S + st * 128: h * S + st * 128 + 128],
                            wo_sb[:, h * HID + eg * 512: h * HID + eg * 512 + 512],
                            start=(h == 0), stop=(h == HPC - 1),
                        )
                    o_sl = ostage[:, eg * 512:(eg + 1) * 512]
                    if eg % 2 == 0 or st % 4 == 3:
                        nc.vector.tensor_copy(o_sl, ps[:])
                    else:
                        nc.scalar.copy(o_sl, ps[:])
                    if last:
                        # drain the very last row tile in eg-sized
                        # pieces so the final transfer is short
                        nc.sync.dma_start(
                            out[st * 128:(st + 1) * 128,
                                eg * 512:(eg + 1) * 512], o_sl,
                        )
                    elif eg == SB - 1:
                        nc.sync.dma_start(
                            out[st * 128:(st + 1) * 128, :], ostage[:]
                        )

                def oproj_units(st):
                    cell = [None]
                    return [lambda eg=eg: oproj_eg(st, eg, cell)
                            for eg in range(SB)]

                def oproj_st(st):
                    for fl in oproj_units(st):
                        fl()

                # ---- schedule ----
                # qproj(0)/qproj(1) run as blocks before attention; later
                # qproj chains and all oproj row tiles are interleaved into
                # the attention loops as PE filler so the exp pipeline
                # latency never idles the PE.  Rope chains slot into
                # per-head DVE gaps (dve_extra) so the next group's
                # masks/accumulates aren't queued behind them.
                q0 = [qproj_head(0, h) for h in range(HPC)]
                vtrans(0, ST)
                for h in range(HPC):
                    rope_rotate(q0[h], 0)
                q1 = [qproj_head(1, h) for h in range(HPC)]
                attn(
                    0,
                    fillers={
                        0: [lambda: qproj_head(2, 0)],
                        1: [lambda: qproj_head(2, 1)],
                        2: [lambda: qproj_head(2, 2)],
                        4: [lambda: qproj_head(2, 3)],
                    },
                    dve_extra={
                        0: [lambda: rope_rotate(q1[0], 1),
                            lambda: rope_rotate(q1[1], 1)],
                        1: [lambda: rope_rotate(q1[2], 1),
                            lambda: rope_rotate(q1[3], 1)],
                        2: [lambda: rope_rotate(kT_sb[:, 512:1024], 1),
                            lambda: rope_rotate(kT_sb[:, 1024:1536], 2)],
                        3: [lambda: rope_rotate(kT_sb[:, 1536:2048], 3)],
                    },
                )
                attn(
                    1,
                    fillers={
                        1: [lambda: qproj_head(3, 2)],
                        2: [lambda: qproj_head(3, 3)],
                        4: [lambda: oproj_st(2)],
                    },
                    # the two fin-independent qproj chains lead the queue so
                    # the first pops (head 0) never wait on fin(0)'s at rows
                    fillq=([lambda: qproj_head(3, 0),
                            lambda: qproj_head(3, 1)]
                           + oproj_units(0) + oproj_units(1)),
                    stride=4,
                    dve_extra={
                        0: [lambda: rope_rotate(qsl(2, 0), 2)],
                        1: [lambda: rope_rotate(qsl(2, 1), 2)],
                        2: [lambda: rope_rotate(qsl(2, 2), 2)],
                        3: [lambda: rope_rotate(qsl(2, 3), 2)],
                    },
                )
                attn(
                    2,
                    fillers={
                        4: [lambda: oproj_st(7)],
                    },
                    fillq=(oproj_units(3) + oproj_units(4)
                           + oproj_units(5) + oproj_units(6)),
                    stride=3,
                    dve_extra={
                        0: [lambda: rope_rotate(qsl(3, 0), 3)],
                        1: [lambda: rope_rotate(qsl(3, 1), 3)],
                        2: [lambda: rope_rotate(qsl(3, 2), 3)],
                        3: [lambda: rope_rotate(qsl(3, 3), 3)],
                    },
                )
                attn(
                    3,
                    fillers={
                        4: [lambda: oproj_st(11)],
                    },
                    fillq=(oproj_units(8) + oproj_units(9)
                           + oproj_units(10)),
                    stride=5,
                    dve_extra={},
                )
                for st in range(12, 16):
                    oproj_st(st)
    _legalize_waits(nc)
    return nc


def _host_prep(hidden_states, Wq, Wk, Wv, Wo, position_ids):
    bf = ml_dtypes.bfloat16
    inv_freq = 1.0 / (THETA ** (np.arange(0, HD, 2, dtype=np.float64) / HD))

    mask = np.zeros((4, 128, 512), dtype=bf)
    jl = np.arange(128)[:, None]
    il = np.arange(512)[None, :]
    for p in range(4):
        mask[p] = (128 * p + jl <= il).astype(bf)
    # flat [128, 4*512] layout matching mask_sb
    mask_flat = np.ascontiguousarray(mask.transpose(1, 0, 2).reshape(128, 4 * 512))

    def flat(w):  # [KT,128,N] chunked -> [128, KT*N] sbuf layout
        return np.ascontiguousarray(w.transpose(1, 0, 2).reshape(128, -1))

    in_maps = []
    for d in range(NCORES):
        b, g = d // 4, d % 4
        hsT = np.ascontiguousarray(hidden_states[b].T).astype(bf).reshape(KT, 128, S)
        wqT = np.ascontiguousarray(Wq[g * QDIM:(g + 1) * QDIM].T).astype(bf).reshape(KT, 128, QDIM)
        wkT = np.ascontiguousarray(Wk[g * HD:(g + 1) * HD].T).astype(bf).reshape(KT, 128, HD)
        wvT = np.ascontiguousarray(Wv[g * HD:(g + 1) * HD].T).astype(bf).reshape(KT, 128, HD)
        woT = np.ascontiguousarray(Wo[:, g * QDIM:(g + 1) * QDIM].T).astype(bf).reshape(4, 128, HID)
        freqs = position_ids[b].astype(np.float64)[:, None] * inv_freq[None, :]  # [S, 64]
        emb = np.concatenate([freqs, freqs], axis=1)  # [S, 128]
        cosT = np.cos(emb).T.astype(bf)
        sinT = np.sin(emb).T.astype(bf)
        in_maps.append({
            "hsT": hsT, "wqT": flat(wqT), "wkT": flat(wkT), "wvT": flat(wvT),
            "woT": flat(woT),
            "cosT": np.ascontiguousarray(cosT),
            "sinT": np.ascontiguousarray(sinT),
            "mask": mask_flat,
        })
    return in_maps


def kernel(hidden_states, Wq, Wk, Wv, Wo, position_ids, _trace=False, _tmpdir=None):
    from concourse.bass_utils import run_bass_kernel_spmd

    if "nc" not in _CACHE:
        _CACHE["nc"] = _build_nc()
    nc = _CACHE["nc"]

    in_maps = _host_prep(
        np.asarray(hidden_states), np.asarray(Wq), np.asarray(Wk),
        np.asarray(Wv), np.asarray(Wo), np.asarray(position_ids),
    )
    res = run_bass_kernel_spmd(
        nc, in_maps, core_ids=list(range(NCORES)), trace=_trace, tmpdir=_tmpdir
    )
    _CACHE["last_result"] = res

    out = np.zeros((B, S, NH * HD), dtype=np.float32)
    for d in range(NCORES):
        out[d // 4] += res.results[d]["out"].astype(np.float32)
    return out



# revision 33
# speedup vs baseline: 1.0350x; 1.0350x over previous
"""Tensor-parallel Llama attention for 8 TRN2 NeuronCores.

Sharding: core d handles batch d//4 and q-head group g = d%4 (q heads
4g..4g+3, kv head g — GQA group-aligned so each core needs exactly one
kv head).  Wq/Wk/Wv are row-sharded, Wo column-sharded; the per-batch
partial o_proj outputs of 4 cores are summed on the host.

Device layouts (prepared host-side, bf16):
  hsT  [16,128,S]   hidden_states[b].T, HID on partitions in 16 chunks
  wqT  [16,128,512] Wq_shard.T          wkT/wvT [16,128,128]
  woT  [4,128,2048] Wo_shard.T (4 contraction chunks of the 512 local dims)
  cosT/sinT [128,S] RoPE tables in [head_dim, seq] layout
  mask [4,128,512]  0/1 causal masks for the 4 diagonal-block phases

Schedule (v4): half-chunks of hs0/wk/wv DMA first so the k-projection
starts as early as possible; K+V projections run contraction-chunk-
outer with 8 PSUM accumulators streaming right behind the hsT DMA.
Attention probs are query-aligned and tree-accumulated 4:1 on the DVE
in bf16, so the softmax-denominator ones-matmuls only stream every
4th tile through the PE (quarter-rate f32 sums — this cut PE busy by
~25us vs per-tile sum matmuls).  qproj(2..3) chains and all oproj row
tiles are interleaved INTO the attention tile loops as per-head PE
filler (fillers[h], emitted after each head's first two scores), and
each group carries a post-filler (key 4) after its last head so the
next group's fin broadcast-matmuls — which wait on the ln/exp
reciprocal chain — always have PE cover.  Rope rotate chains slot
into per-head DVE gaps (dve_extra) so masks/tree-adds aren't queued
behind them.  Diagonal blocks are width-trimmed to the causal
triangle with a 2-tile score lookahead.  Denominators are exp(-ln(s))
on ACT; pv*(1/sum) is fused into one scalar_tensor_tensor.  o_proj
stages whole 128x2048 bf16 rows, one DMA per row tile; partial
outputs are summed on the host in f32.  (fp8/DoubleRow was evaluated
and rejected: e4m3's ~3% GEMM error exceeds the 2e-2 gate on every
path except qproj, and even there leaves no margin.)
"""

import sys

sys.path.insert(0, "/opt/trn_rl_repo")

import numpy as np
import ml_dtypes

B, S, HID = 2, 2048, 2048
NH, NKV, HD = 16, 4, 128
THETA = 10000.0
NCORES = 8
HPC = 4            # q heads per core
QDIM = HPC * HD    # 512 local q dims
KT = HID // 128    # 16 contraction chunks
SB = S // 512      # 4 column groups of 512
ST = S // 128      # 16 row tiles of 128

_CACHE = {}


def _patch_tile_drain():
    """This walrus build caps sync waits per CTRL instruction below what the
    stock Tile kernel-tail drain carries; split them into single-wait NOPs."""
    import bass_rust
    import concourse.tile as tile
    from concourse.tile import ScopedClock

    if getattr(tile.TileContext, "_drain_split_patched", False):
        return

    def _split_drain_and_barrier(self, tick_clock, wait_clock):
        ticks = list(tick_clock.global_clock)
        for i, v in enumerate(ticks):
            if v > 0:
                single = [0] * len(ticks)
                single[i] = v
                nop = self.nc.sync.nop(nofuse=True, hint=f"drain_wait_{i}")
                wait_clock.add_sem_waits(
                    nop.ins, ScopedClock({None: bass_rust.VectorClock(single)})
                )
        self.nc.sync.drain()
        self.nc.all_engine_barrier()
        assert self.sems is not None
        popped = self.nc._tile_sem_poison_stack.pop()
        assert popped is self._sem_poison
        self.nc.clear_and_free_semaphores(list(self.sems.allocated().values()))
        self.nc.all_engine_barrier()

    tile.TileContext._drain_and_barrier = _split_drain_and_barrier
    tile.TileContext._drain_split_patched = True


def _legalize_waits(nc, max_waits=1):
    """This walrus build rejects instructions carrying more than ~2 sync
    waits.  Hoist the excess onto single-wait NOPs inserted just before the
    instruction in its block (same engine => same instruction stream, so
    the waits still complete before the op issues)."""
    import concourse.mybir as mybir

    n_split = 0
    for block in nc.m.functions[0].blocks:
        insts = list(block.instructions)
        out = []
        for inst in insts:
            si = getattr(inst, "sync_info", None)
            if si is not None and si.on_wait and len(si.on_wait) > max_waits:
                waits = list(si.on_wait)
                keep = waits[:max_waits]
                for j, w in enumerate(waits[max_waits:]):
                    out.append(
                        mybir.InstNoOp(
                            name=f"{inst.name}_hw{j}",
                            engine=inst.engine,
                            bass_nofuse=True,
                            sync_info=mybir.SyncInfo(on_wait=[w], on_update=[]),
                        )
                    )
                si.on_wait = keep
                n_split += 1
            out.append(inst)
        block.instructions = out
    return n_split


def _build_nc():
    import concourse.bass as bass
    import concourse.mybir as mybir
    import concourse.tile as tile
    from concourse.masks import make_identity

    _patch_tile_drain()

    bf = mybir.dt.bfloat16
    f32 = mybir.dt.float32
    Exp = mybir.ActivationFunctionType.Exp
    Ln = mybir.ActivationFunctionType.Ln
    Mult = mybir.AluOpType.mult

    nc = bass.Bass()
    hsT = nc.declare_dram_parameter("hsT", [KT, 128, S], bf, isOutput=False)
    wqT = nc.declare_dram_parameter("wqT", [128, KT * QDIM], bf, isOutput=False)
    wkT = nc.declare_dram_parameter("wkT", [128, KT * HD], bf, isOutput=False)
    wvT = nc.declare_dram_parameter("wvT", [128, KT * HD], bf, isOutput=False)
    woT = nc.declare_dram_parameter("woT", [128, 4 * HID], bf, isOutput=False)
    cosT = nc.declare_dram_parameter("cosT", [128, S], bf, isOutput=False)
    sinT = nc.declare_dram_parameter("sinT", [128, S], bf, isOutput=False)
    mask = nc.declare_dram_parameter("mask", [128, 4 * 512], bf, isOutput=False)
    out = nc.declare_dram_parameter("out", [S, HID], bf, isOutput=True)

    inv_sqrt_d = 1.0 / float(np.sqrt(HD))

    with tile.TileContext(nc) as tc:
        with (
            tc.tile_pool(name="resid", bufs=1) as resid,
            tc.tile_pool(name="probs", bufs=10) as probs_pool,
            tc.tile_pool(name="rot", bufs=2) as rot_pool,
            tc.tile_pool(name="rcp", bufs=4) as rcp_pool,
            tc.tile_pool(name="ostage", bufs=3) as ostage_pool,
        ):
            hs_sb = resid.tile([128, KT * S], bf)
            wq_sb = resid.tile([128, KT * QDIM], bf)
            wk_sb = resid.tile([128, KT * HD], bf)
            wv_sb = resid.tile([128, KT * HD], bf)
            wo_sb = resid.tile([128, 4 * HID], bf)
            cos_sb = resid.tile([128, S], bf)
            sin_sb = resid.tile([128, S], bf)
            mask_sb = resid.tile([128, 4 * 512], bf)
            ones_sb = resid.tile([128, 1], bf)
            ones128 = resid.tile([128, 128], bf)
            ident = resid.tile([128, 128], bf)
            qT_sb = resid.tile([128, HPC * S], bf)
            kT_sb = resid.tile([128, S], bf)
            vT_sb = resid.tile([128, S], bf)
            vn_sb = resid.tile([128, S], bf)
            at_sb = resid.tile([128, HPC * S], bf)

            # ---- loads: hs chunk 0 goes FIRST so the k-projection's first
            # matmul can issue at ~2.5us (wk arrives right behind it); the
            # wq/cos/sin/mask transfers are slotted into the hs stream at
            # the point their consumers need them ----
            QC = 4 * QDIM
            # half-chunks of hs0/wk/wv go first so the k-projection's
            # opening matmuls start as soon as ~512KB has landed; the
            # wq/cos/sin/mask transfers are slotted into the hs stream at
            # the point their consumers need them
            nc.sync.dma_start(hs_sb[:, 0:1024], hsT[0][:, 0:1024])
            nc.sync.dma_start(wk_sb[:, 0:1024], wkT[:, 0:1024])
            nc.sync.dma_start(wv_sb[:, 0:1024], wvT[:, 0:1024])
            nc.sync.dma_start(hs_sb[:, 1024:S], hsT[0][:, 1024:S])
            nc.sync.dma_start(wk_sb[:, 1024:2048], wkT[:, 1024:2048])
            nc.sync.dma_start(wv_sb[:, 1024:2048], wvT[:, 1024:2048])
            for kk in range(1, 8):
                nc.sync.dma_start(hs_sb[:, kk * S:(kk + 1) * S], hsT[kk])
            nc.sync.dma_start(wq_sb[:, 0:QC], wqT[:, 0:QC])
            for kk in range(8, 12):
                nc.sync.dma_start(hs_sb[:, kk * S:(kk + 1) * S], hsT[kk])
            nc.sync.dma_start(cos_sb[:], cosT[:])
            nc.sync.dma_start(sin_sb[:], sinT[:])
            for kk in range(12, KT):
                nc.sync.dma_start(hs_sb[:, kk * S:(kk + 1) * S], hsT[kk])
            nc.sync.dma_start(mask_sb[:], mask[:])
            for c in range(1, 4):
                nc.sync.dma_start(wq_sb[:, c * QC:(c + 1) * QC], wqT[:, c * QC:(c + 1) * QC])
            nc.sync.dma_start(wo_sb[:], woT[:])
            nc.gpsimd.memset(ones_sb[:], 1.0)
            nc.gpsimd.memset(ones128[:], 1.0)
            make_identity(nc, ident[:])

            def rope_rotate(dst, sg):
                # in-place rope of dst (sbuf bf16 [128,512]); sg picks the
                # cos/sin column group.
                cs = cos_sb[:, sg * 512:(sg + 1) * 512]
                sn = sin_sb[:, sg * 512:(sg + 1) * 512]
                rot = rot_pool.tile([128, 512], bf, name="rot", tag="rot")
                nc.vector.tensor_scalar_mul(rot[0:64, :], dst[64:128, :], -1.0)
                nc.vector.tensor_copy(rot[64:128, :], dst[0:64, :])
                nc.vector.tensor_mul(dst, dst, cs)
                nc.vector.tensor_mul(rot[:], rot[:], sn)
                nc.vector.tensor_add(dst, dst, rot[:])


            # ---- k/v projections, contraction-chunk outer ----
            with tc.tile_pool(name="ldps", bufs=8, space="PSUM") as ldps:
                ldk = [ldps.tile([128, 512], f32, name=f"ldk{sg}", tag="ld")
                       for sg in range(SB)]
                ldv = [ldps.tile([128, 512], f32, name=f"ldv{sg}", tag="ld")
                       for sg in range(SB)]
                for kk in range(KT):
                    for sg in range(SB):
                        nc.tensor.matmul(
                            ldk[sg][:],
                            wk_sb[:, kk * HD: kk * HD + 128],
                            hs_sb[:, kk * S + sg * 512: kk * S + sg * 512 + 512],
                            start=(kk == 0), stop=(kk == KT - 1),
                        )
                        nc.tensor.matmul(
                            ldv[sg][:],
                            wv_sb[:, kk * HD: kk * HD + 128],
                            hs_sb[:, kk * S + sg * 512: kk * S + sg * 512 + 512],
                            start=(kk == 0), stop=(kk == KT - 1),
                        )
                # v copies on ACT first so the PE transposes can start while
                # the DVE is still busy with rope(k).
                for sg in range(SB):
                    nc.scalar.copy(vT_sb[:, sg * 512:(sg + 1) * 512], ldv[sg][:])
                for sg in range(SB):
                    nc.vector.tensor_copy(kT_sb[:, sg * 512:(sg + 1) * 512], ldk[sg][:])
                # attn(0) only touches j-tiles 0-3 (= sg0): rotate that one
                # now, defer sg1-3 until after qproj(0)'s rotates
                rope_rotate(kT_sb[:, 0:512], 0)

            with (
                tc.tile_pool(name="tr_ps", bufs=3, space="PSUM") as tr_ps,
                tc.tile_pool(name="pv_ps", bufs=4, space="PSUM") as pv_ps,
                tc.tile_pool(name="sm_ps", bufs=1, space="PSUM") as sm_ps,
            ):
                def vtrans(tj0, tj1):
                    # ---- v back to natural [s, d] layout via PE transpose ----
                    for tj in range(tj0, tj1):
                        tp = tr_ps.tile([128, 128], bf, name="tp", tag="tr")
                        nc.tensor.transpose(tp[:], vT_sb[:, tj * 128:(tj + 1) * 128], ident[:])
                        nc.vector.tensor_copy(vn_sb[:, tj * 128:(tj + 1) * 128], tp[:])

                def qsl(gi, h):
                    return qT_sb[:, h * S + gi * 512: h * S + gi * 512 + 512]

                def qproj_head(gi, h):
                    # one 16-matmul accumulation chain; psum copy on ACT so
                    # the DVE stays free for rope/mask/accumulate work
                    ps = tr_ps.tile([128, 512], f32, name="qps", tag="tr")
                    for kk in range(KT):
                        nc.tensor.matmul(
                            ps[:],
                            wq_sb[:, kk * QDIM + h * 128: kk * QDIM + (h + 1) * 128],
                            hs_sb[:, kk * S + gi * 512: kk * S + gi * 512 + 512],
                            start=(kk == 0), stop=(kk == KT - 1),
                        )
                    sl = qsl(gi, h)
                    nc.scalar.copy(sl, ps[:])
                    return sl

                def attn(gi, pre=None, fillers=None, dve_extra=None,
                         fillq=None, stride=4):
                    fillq = fillq if fillq is not None else []
                    # Per-head tiles, query-aligned: tile t's valid columns
                    # are [ioff:512] (ioff=128p on the 4 diagonal tiles, 0
                    # elsewhere).  Probs tree-accumulate 4:1 on the DVE in
                    # bf16 and only the group-of-4 roots stream through the
                    # ones-matmul into smp (quarter-rate f32 sums).
                    # pre() is emitted after head 0's first two scores;
                    # fillers[h] (PE chains) after head h's first two
                    # scores; dve_extra[h] (DVE work) after head h's tiles.
                    tiles = []
                    for tj in range(4 * gi):
                        tiles.append((tj, 0, None))
                    for p in range(4):
                        tiles.append((4 * gi + p, 128 * p, p))
                    ntile = len(tiles)
                    ngrp = gi + 1

                    smp = sm_ps.tile([128, 512], f32, name="smp", tag="sm")
                    pvs = []

                    def emit_head(h):
                        qh = qsl(gi, h)
                        pv = pv_ps.tile([128, 512], f32, name="pv", tag="pv")
                        pvs.append(pv)
                        # software pipeline with one-tile lookahead: emit
                        # score(t+1) before pv(t) so exp overlaps PE.
                        pbs = [None] * ntile
                        # sum matmuls are deferred one tile past their
                        # group's last DVE tree-add so the PE never waits
                        # on the add chain
                        pend_sums = []

                        def flush_sums(now_t):
                            while pend_sums and (
                                now_t is None or now_t >= pend_sums[0][1] + 4
                            ):
                                g, t0 = pend_sums.pop(0)
                                nc.tensor.matmul(
                                    smp[32 * h:32 * h + 1, 0:512],
                                    ones_sb[:], pbs[t0][:, 0:512],
                                    start=(g == 0), stop=(g == ngrp - 1),
                                    skip_group_check=True,
                                    tile_position=(0, 32 * h),
                                )

                        def emit_score(t):
                            tj, ioff, p = tiles[t]
                            sc = tr_ps.tile([128, 512], f32, name="sc", tag="tr")
                            nc.tensor.matmul(
                                sc[:, ioff:512],
                                kT_sb[:, tj * 128:(tj + 1) * 128],
                                qh[:, ioff:512],
                                start=True, stop=True,
                            )
                            pb = probs_pool.tile([128, 512], bf, name="pb")
                            nc.scalar.activation(
                                pb[:, ioff:512], sc[:, ioff:512], Exp,
                                scale=inv_sqrt_d,
                            )
                            if p is not None:
                                nc.vector.tensor_mul(
                                    pb[:, ioff:512], pb[:, ioff:512],
                                    mask_sb[:, p * 512 + ioff:(p + 1) * 512],
                                )
                            pbs[t] = pb

                        def emit_sumpv(t):
                            tj, ioff, p = tiles[t]
                            nc.tensor.matmul(
                                pv[:, ioff:512],
                                vn_sb[:, tj * 128:(tj + 1) * 128],
                                pbs[t][:, ioff:512],
                                start=(t == 0), stop=(t == ntile - 1),
                                skip_group_check=True,
                            )
                            g, r = divmod(t, 4)
                            t0 = 4 * g
                            if r == 1:
                                io = tiles[t][1]
                                nc.vector.tensor_add(
                                    pbs[t0][:, io:512], pbs[t0][:, io:512],
                                    pbs[t][:, io:512],
                                )
                            elif r == 3:
                                io = tiles[t][1]
                                nc.vector.tensor_add(
                                    pbs[t0 + 2][:, io:512],
                                    pbs[t0 + 2][:, io:512], pbs[t][:, io:512],
                                )
                                io2 = tiles[t0 + 2][1]
                                nc.vector.tensor_add(
                                    pbs[t0][:, io2:512], pbs[t0][:, io2:512],
                                    pbs[t0 + 2][:, io2:512],
                                )
                                pend_sums.append((g, t0))

                        emit_score(0)
                        if ntile > 1:
                            emit_score(1)
                        if h == 0 and pre is not None:
                            pre()
                        if fillers is not None:
                            for fl in fillers.get(h, []):
                                fl()
                        for t in range(ntile):
                            if t + 2 < ntile:
                                emit_score(t + 2)
                            emit_sumpv(t)
                            flush_sums(t)
                            # fine-grained PE filler inside the tile loop:
                            # the ACT exp runs ~130ns/tile slower than the
                            # PE's score+pv work, and the 3-slot score psum
                            # ring caps its lookahead, so without these the
                            # PE stalls on pb mid-head.
                            if fillq and t >= 1 and t % stride == stride - 1:
                                fillq.pop(0)()
                        flush_sums(None)
                        if dve_extra is not None:
                            for fn in dve_extra.get(h, []):
                                fn()

                    for h in range(HPC):
                        emit_head(h)
                    # drain any filler units the tile loops didn't consume
                    while fillq:
                        fillq.pop(0)()
                    # batched 1/sums for all 4 heads (rows 32h of smp) as
                    # exp(-ln(s)) on ACT: ln/exp/copy share one act table, and
                    # this is ~2.6us cheaper than the DVE reciprocal.  The
                    # per-head broadcast+normalize follows in fin(gi), which
                    # the caller schedules behind PE filler work.
                    lns = rcp_pool.tile([128, 512], f32, name="lns", tag="rcp")
                    nc.scalar.activation(lns[:], smp[:], Ln)
                    rcpb = rcp_pool.tile([128, 512], bf, name="rcpb", tag="rcpb")
                    nc.scalar.activation(rcpb[:], lns[:], Exp, scale=-1.0)
                    # post-filler slot (key 4): PE work emitted after the
                    # last head so this group's own fin broadcast matmuls
                    # (which wait on the ln/exp chain) have PE cover.
                    if fillers is not None:
                        for fl in fillers.get(4, []):
                            fl()
                    # normalize in-group: by now the reciprocals are ready
                    # (post-filler covered the ACT latency), the tr_ps ring
                    # has free slots, and finishing here releases the pv
                    # banks before the next group allocates them.
                    fin(gi, pvs, rcpb)

                def fin(gi, pvs, rcpb):
                    for h in range(HPC):
                        bc = tr_ps.tile([128, 512], f32, name="bc", tag="tr")
                        nc.tensor.matmul(
                            bc[:], ones128[32 * h:32 * h + 1, :],
                            rcpb[32 * h:32 * h + 1, :],
                            start=True, stop=True,
                            tile_position=(32 * h, 0),
                        )
                        bcs = rcp_pool.tile([128, 512], bf, name="bcs", tag="bcs")
                        nc.scalar.copy(bcs[:], bc[:])
                        a_sl = at_sb[:, h * S + gi * 512: h * S + gi * 512 + 512]
                        nc.vector.scalar_tensor_tensor(
                            a_sl, pvs[h][:], 1.0, bcs[:], Mult, Mult
                        )

                def oproj_eg(st, eg, cell):
                    # one eg-chain of an output row tile (4 matmuls + copy);
                    # the ostage tile is allocated on eg 0 and the row DMA
                    # issued on eg 3, so these units can be sprinkled into
                    # the attention tile loops as fine-grained PE filler
                    if eg == 0:
                        cell[0] = ostage_pool.tile([128, HID], bf, name="ostage")
                    ostage = cell[0]
                    last = st == 15
                    ps = tr_ps.tile([128, 512], f32, name="ops", tag="tr")
                    for h in range(HPC):
                        nc.tensor.matmul(
                            ps[:],
                            at_sb[:, h * S + st * 128: h * S + st * 128 + 128],
                            wo_sb[:, h * HID + eg * 512: h * HID + eg * 512 + 512],
                            start=(h == 0), stop=(h == HPC - 1),
                        )
                    o_sl = ostage[:, eg * 512:(eg + 1) * 512]
                    if eg % 2 == 0 or st % 4 == 3:
                        nc.vector.tensor_copy(o_sl, ps[:])
                    else:
                        nc.scalar.copy(o_sl, ps[:])
                    if last:
                        # drain the very last row tile in eg-sized
                        # pieces so the final transfer is short
                        nc.sync.dma_start(
                            out[st * 128:(st + 1) * 128,
                                eg * 512:(eg + 1) * 512], o_sl,
                        )
                    elif eg == SB - 1:
                        nc.sync.dma_start(
                            out[st * 128:(st + 1) * 128, :], ostage[:]
                        )

                def oproj_units(st):
                    cell = [None]
                    return [lambda eg=eg: oproj_eg(st, eg, cell)
                            for eg in range(SB)]

                def oproj_st(st):
                    for fl in oproj_units(st):
                        fl()

                # ---- schedule ----
                # qproj(0)/qproj(1) run as blocks before attention; later
                # qproj chains and all oproj row tiles are interleaved into
                # the attention loops as PE filler so the exp pipeline
                # latency never idles the PE.  Rope chains slot into
                # per-head DVE gaps (dve_extra) so the next group's
                # masks/accumulates aren't queued behind them.
                q0 = [qproj_head(0, h) for h in range(HPC)]
                vtrans(0, ST)
                for h in range(HPC):
                    rope_rotate(q0[h], 0)
                q1 = [qproj_head(1, h) for h in range(HPC)]
                attn(
                    0,
                    fillers={
                        0: [lambda: qproj_head(2, 0)],
                        1: [lambda: qproj_head(2, 1)],
                        2: [lambda: qproj_head(2, 2)],
                        4: [lambda: qproj_head(2, 3)],
                    },
                    dve_extra={
                        0: [lambda: rope_rotate(q1[0], 1),
                            lambda: rope_rotate(q1[1], 1)],
                        1: [lambda: rope_rotate(q1[2], 1),
                            lambda: rope_rotate(q1[3], 1)],
                        2: [lambda: rope_rotate(kT_sb[:, 512:1024], 1),
                            lambda: rope_rotate(kT_sb[:, 1024:1536], 2)],
                        3: [lambda: rope_rotate(kT_sb[:, 1536:2048], 3)],
                    },
                )
                attn(
                    1,
                    fillers={
                        1: [lambda: qproj_head(3, 2)],
                        2: [lambda: qproj_head(3, 3)],
                        4: [lambda: oproj_st(2)],
                    },
                    # the two fin-independent qproj chains lead the queue so
                    # the first pops (head 0) never wait on fin(0)'s at rows
                    fillq=([lambda: qproj_head(3, 0),
                            lambda: qproj_head(3, 1)]
                           + oproj_units(0) + oproj_units(1)),
                    stride=4,
                    dve_extra={
                        0: [lambda: rope_rotate(qsl(2, 0), 2)],
                        1: [lambda: rope_rotate(qsl(2, 1), 2)],
                        2: [lambda: rope_rotate(qsl(2, 2), 2)],
                        3: [lambda: rope_rotate(qsl(2, 3), 2)],
                    },
                )
                attn(
                    2,
                    fillers={
                        4: [lambda: oproj_st(7)],
                    },
                    fillq=(oproj_units(3) + oproj_units(4)
                           + oproj_units(5) + oproj_units(6)),
                    stride=3,
                    dve_extra={
                        0: [lambda: rope_rotate(qsl(3, 0), 3)],
                        1: [lambda: rope_rotate(qsl(3, 1), 3)],
                        2: [lambda: rope_rotate(qsl(3, 2), 3)],
                        3: [lambda: rope_rotate(qsl(3, 3), 3)],
                    },
                )
                attn(
                    3,
                    fillers={
                        4: [lambda: oproj_st(11)],
                    },
                    fillq=(oproj_units(8) + oproj_units(9)
                           + oproj_units(10)),
                    stride=5,
                    dve_extra={},
                )
                for st in range(12, 16):
                    oproj_st(st)
    _legalize_waits(nc)
    return nc


def _host_prep(hidden_states, Wq, Wk, Wv, Wo, position_ids):
    bf = ml_dtypes.bfloat16
    inv_freq = 1.0 / (THETA ** (np.arange(0, HD, 2, dtype=np.float64) / HD))

    mask = np.zeros((4, 128, 512), dtype=bf)
    jl = np.arange(128)[:, None]
    il = np.arange(512)[None, :]
    for p in range(4):
        mask[p] = (128 * p + jl <= il).astype(bf)
    # flat [128, 4*512] layout matching mask_sb
    mask_flat = np.ascontiguousarray(mask.transpose(1, 0, 2).reshape(128, 4 * 512))

    def flat(w):  # [KT,128,N] chunked -> [128, KT*N] sbuf layout
        return np.ascontiguousarray(w.transpose(1, 0, 2).reshape(128, -1))

    in_maps = []
    for d in range(NCORES):
        b, g = d // 4, d % 4
        hsT = np.ascontiguousarray(hidden_states[b].T).astype(bf).reshape(KT, 128, S)
        wqT = np.ascontiguousarray(Wq[g * QDIM:(g + 1) * QDIM].T).astype(bf).reshape(KT, 128, QDIM)
        wkT = np.ascontiguousarray(Wk[g * HD:(g + 1) * HD].T).astype(bf).reshape(KT, 128, HD)
        wvT = np.ascontiguousarray(Wv[g * HD:(g + 1) * HD].T).astype(bf).reshape(KT, 128, HD)
        woT = np.ascontiguousarray(Wo[:, g * QDIM:(g + 1) * QDIM].T).astype(bf).reshape(4, 128, HID)
        freqs = position_ids[b].astype(np.float64)[:, None] * inv_freq[None, :]  # [S, 64]
        emb = np.concatenate([freqs, freqs], axis=1)  # [S, 128]
        cosT = np.cos(emb).T.astype(bf)
        sinT = np.sin(emb).T.astype(bf)
        in_maps.append({
            "hsT": hsT, "wqT": flat(wqT), "wkT": flat(wkT), "wvT": flat(wvT),
            "woT": flat(woT),
            "cosT": np.ascontiguousarray(cosT),
            "sinT": np.ascontiguousarray(sinT),
            "mask": mask_flat,
        })
    return in_maps


def kernel(hidden_states, Wq, Wk, Wv, Wo, position_ids, _trace=False, _tmpdir=None):
    from concourse.bass_utils import run_bass_kernel_spmd

    if "nc" not in _CACHE:
        _CACHE["nc"] = _build_nc()
    nc = _CACHE["nc"]

    in_maps = _host_prep(
        np.asarray(hidden_states), np.asarray(Wq), np.asarray(Wk),
        np.asarray(Wv), np.asarray(Wo), np.asarray(position_ids),
    )
    res = run_bass_kernel_spmd(
        nc, in_maps, core_ids=list(range(NCORES)), trace=_trace, tmpdir=_tmpdir
    )
    _CACHE["last_result"] = res

    out = np.zeros((B, S, NH * HD), dtype=np.float32)
    for d in range(NCORES):
        out[d // 4] += res.results[d]["out"].astype(np.float32)
    return out

